# revision 36
# baseline (speedup 1.0000x reference)
"""CARFAC cell kernel for 8 TRN2 NeuronCores.

Math: y[b,c,n] is the linear recurrence a[n+1] = f[n+1]*a[n] + g[n+1]
(computed exactly with the DVE's tensor_tensor_scan instruction — the
reference's cumsum-of-logs + triangular-matmul expansion is just a
parallel-friendly expression of the same recurrence), followed by
`steps` rounds of a symmetric-padded 3-tap FIR across channels.

Key identity for the smoothing stage: half-sample symmetric padding
commutes with a symmetric FIR, so applying the 3-tap kernel `steps`
times equals ONE conv with the `steps`-fold self-convolution of the
kernel (17 taps for steps=8) on the reflect-extended signal. That
collapses to a single [C x C] matrix W (banded + boundary-folded),
i.e. one TensorEngine matmul.

Sharding: 8 cores = 2 batches x 4 channel-quarters. Each core loads its
owned ~18 channels plus an 8-channel halo (<=34 rows of f/g), scans the
recurrence for all loaded rows, and applies its [34 x 18] slice of W
(halo selection + reflection encoded host-side in the weights). No
cross-core communication of any kind.

Performance notes (from neuron-profile traces):
- A dynamic DMA's descriptors are processed by ONE SDMA engine
  (~27 GB/s = one SBUF port); the sync, scalar and gpsimd DGE paths
  are independent, so transfers are split across all three.
- The input is loaded in three waves matched to the scan chunks, so the
  scan streams behind the loads instead of waiting for the full input.
- Raw Bass (no Tile, no Block): Tile's tail drain exceeds the HW's
  per-instruction sync-wait cap, and Block's exit all-engine barrier
  costs ~4 us of pure epilogue.
- FP32r single-pass matmuls (4x the fp32 PE rate); end-to-end relative
  error vs the fp32 reference is ~1.9e-4 (fp32 matmuls give ~3e-6 at
  ~+1.5 us — flip FP32R off if a tighter tolerance is ever needed).
"""

import numpy as np

B, C, N = 2, 71, 1024
NCORES = 8
QPB = 4  # channel-quarters per batch element
HALO = 8  # channel reach of the smoothing: steps * (ksz-1)//2
ROWS = 30  # rows loaded per core: own + halo (ownership balanced so every
#            quarter loads the same row count despite one-sided edge halos)
OWN = 22  # max owned output channels per core

_OWN_LO = [0, 22, 36, 50]
_OWN_SZ = [22, 14, 14, 21]

HALF = 512
CL = 256  # scan/matmul pipeline chunk length (Q = 4 chunks)
# Packed input in three DMA waves matched to scan chunks, so the scan
# streams behind the loads (wave 2 carries two chunks: 3 waves is the
# most the per-ring descriptor-generation rate sustains):
#   wave0: [f q0 | g q0 | a0 | W]   cols [0, 531)
#   wave1: [f q1 | g q1]            cols [531, 1043)
#   wave2: [f q2 | f q3 | g q2 | g q3]  cols [1043, 2067)
_A0 = 2 * CL  # 512
_W0 = _A0 + 1
_WAVES = [0, 2 * CL + 1 + OWN, 4 * CL + 1 + OWN, 8 * CL + 1 + OWN]  # boundaries
_FCOL = [0, _WAVES[1], _WAVES[2], _WAVES[2] + CL]
_GCOL = [CL, _WAVES[1] + CL, _WAVES[2] + 2 * CL, _WAVES[2] + 3 * CL]
PACK = _WAVES[3]

FP32R = True  # single-pass PE matmul (4x faster); flip off if accuracy drops
FINAL_WAITS = False  # engines wait for output-DMA completion before halting

_PROGRAM = None


def _build_program():
    import concourse.bass as bass
    import concourse.mybir as mybir

    f32 = mybir.dt.float32
    mm_dt = mybir.dt.float32r if FP32R else f32
    mult, add = mybir.AluOpType.mult, mybir.AluOpType.add
    nc = bass.Bass(enable_partition_id=False)
    in_loc = nc.declare_dram_parameter("in_loc", [ROWS, PACK], f32, isOutput=False)
    out_loc = nc.declare_dram_parameter("out_loc", [OWN, N], f32, isOutput=True)

    Q = 4  # scan/matmul pipeline chunks

    from contextlib import ExitStack

    with ExitStack() as ctx:
        it = ctx.enter_context(nc.sbuf_tensor([ROWS, PACK], f32))
        yt = ctx.enter_context(nc.sbuf_tensor([ROWS, N], mm_dt))
        wr = ctx.enter_context(nc.sbuf_tensor([ROWS, OWN], mm_dt))
        ot = ctx.enter_context(nc.sbuf_tensor([OWN, N], f32))
        ps = [
            ctx.enter_context(nc.psum_tensor(f"ps{q}", [OWN, CL], f32))
            for q in range(4)
        ]
        sem = lambda name: ctx.enter_context(nc.semaphore(name))
        wave_hw = [sem(f"w{w}_hw") for w in range(3)]  # per-wave HWDGE
        wave_sw = [sem(f"w{w}_sw") for w in range(3)]  # per-wave SWDGE
        o_hw = sem("o_hw")  # output stores
        o_sw = sem("o_sw")
        v_sem = sem("v_sem")  # DVE scans
        p_sem = sem("p_sem")  # PE matmuls
        c_sem = sem("c_sem")  # ACT PSUM->SBUF copies
        w_sem = sem("w_sem")  # W staged as FP32r
        w0_hw, w0_sw = wave_hw[0], wave_sw[0]

        a0t = it[:, _A0 : _A0 + 1]
        wt = it[:, _W0 : _W0 + OWN]

        # Input load: three waves, each split across the three DGE paths
        # (rows balanced by measured per-ring gen+transfer rates).
        SPLITS = [("sync", 0, 11), ("scalar", 11, 20), ("gpsimd", 20, 30)]
        for w in range(3):
            c0, c1 = _WAVES[w], _WAVES[w + 1]
            for eng, r0, r1 in SPLITS:
                getattr(nc, eng).dma_start(
                    out=it[r0:r1, c0:c1], in_=in_loc[r0:r1, c0:c1]
                ).then_inc(wave_sw[w] if eng == "gpsimd" else wave_hw[w], 16)

        # ACT: stage W as FP32r (the PE mode needs FP32r producers). Doing
        # this on ACT keeps it off the DVE critical path, and the first
        # ACTIVATE also absorbs the one-time ~1.3us ACT table load before
        # the PSUM-evacuation copies need it.
        nc.scalar.wait_ge(w0_hw, 32)
        nc.scalar.wait_ge(w0_sw, 16)
        nc.scalar.copy(wr[:, :], wt).then_inc(w_sem, 1)

        # DVE: the recurrence scan in Q chunks, chained via
        # initial=prev_out[:, -1:], streaming behind the input waves.
        for q in range(Q):
            t0, t1 = q * CL, (q + 1) * CL
            if q < 3:  # chunk 3's data arrives with wave 2
                nc.vector.wait_ge(wave_hw[q], 32)
                nc.vector.wait_ge(wave_sw[q], 16)
            init = a0t if q == 0 else yt[:, t0 - 1 : t0]
            if q:
                nc.vector.wait_ge(v_sem, q)  # carry readable (race det.)
            nc.vector.tensor_tensor_scan(
                yt[:, t0:t1],
                it[:, _FCOL[q] : _FCOL[q] + CL],
                it[:, _GCOL[q] : _GCOL[q] + CL],
                init,
                op0=mult,
                op1=add,
            ).then_inc(v_sem, 1)

        # PE: one smoothing matmul per chunk.
        nc.tensor.wait_ge(w_sem, 1)
        for q in range(Q):
            nc.tensor.wait_ge(v_sem, q + 1)
            nc.tensor.matmul(
                ps[q][:, :],
                wr[:, :],
                yt[:, q * CL : (q + 1) * CL],
                start=True,
                stop=True,
            ).then_inc(p_sem, 1)

        # ACT: evacuate PSUM per chunk (runs parallel to later scans).
        for q in range(Q):
            nc.scalar.wait_ge(p_sem, q + 1)
            nc.scalar.copy(ot[:, q * CL : (q + 1) * CL], ps[q][:, :]).then_inc(
                c_sem, 1
            )

        # Stores per half on the sync + gpsimd rings (scalar is busy
        # copying). Halves, not quarters: descriptor generation (~80 ns/row)
        # serializes per ring, so fewer/larger stores win.
        for h in range(2):
            cols = slice(h * HALF, (h + 1) * HALF)
            for eng, r0, r1 in (("sync", 0, 11), ("gpsimd", 11, 22)):
                e = getattr(nc, eng)
                e.wait_ge(c_sem, 2 * (h + 1))
                e.dma_start(out=out_loc[r0:r1, cols], in_=ot[r0:r1, cols]).then_inc(
                    o_sw if eng == "gpsimd" else o_hw, 16
                )
        if FINAL_WAITS:
            nc.sync.wait_ge(o_hw, 32)
            nc.gpsimd.wait_ge(o_sw, 32)

    return nc


def _strip_framework_preamble(nc):
    """Drop the framework preamble's const memsets, engine drains and the
    all-engine EVSEM barrier (~4 us on the critical path). Everything in
    this kernel is gated on data semaphores, so engines starting skewed is
    fine. Serialization-level: patches this instance's to_json_bytes."""
    import orjson

    m = nc.to_json()
    for fn in m["functions"]:
        for blk in fn["blocks"]:
            blk["instructions"] = [
                i
                for i in blk["instructions"]
                if not (
                    i.get("opcode") in ("Memset", "Drain")
                    or str(i.get("name", "")).startswith("barrier_")
                )
            ]
    payload = orjson.dumps(m)
    nc.to_json_bytes = lambda: payload
    return nc


def _conv_matrix(kernel: np.ndarray, steps: int) -> np.ndarray:
    """[C, C] matrix equivalent to `steps` rounds of symmetric-pad conv."""
    eff = np.array([1.0], np.float64)
    for _ in range(steps):
        eff = np.convolve(eff, kernel.astype(np.float64))
    h = (len(eff) - 1) // 2
    assert h <= HALO, f"kernel reach {h} exceeds layout halo {HALO}"
    W = np.zeros((C, C), np.float64)
    for c in range(C):
        for d in range(-h, h + 1):
            idx = c + d
            if idx < 0:
                idx = -1 - idx
            if idx >= C:
                idx = 2 * C - 1 - idx
            W[idx, c] += eff[d + h]
    return W.astype(np.float32)


def _pack_core(core: int, a_0, f, g, W):
    """Build one core's packed [ROWS, PACK] input; returns (in_loc, b, lo, sz)."""
    b, q = divmod(core, QPB)
    lo, sz = _OWN_LO[q], _OWN_SZ[q]
    r0 = max(0, lo - HALO)
    r1 = min(C, lo + sz + HALO)
    nr = r1 - r0

    in_loc = np.zeros((ROWS, PACK), np.float32)
    for q in range(4):
        in_loc[:, _FCOL[q] : _FCOL[q] + CL] = 0.5  # benign f for padded rows
        in_loc[:nr, _FCOL[q] : _FCOL[q] + CL] = f[b, r0:r1, q * CL : (q + 1) * CL]
        in_loc[:nr, _GCOL[q] : _GCOL[q] + CL] = g[b, r0:r1, q * CL : (q + 1) * CL]
    in_loc[:nr, _A0] = a_0[b, r0:r1]
    in_loc[:nr, _W0 : _W0 + sz] = W[r0:r1, lo : lo + sz]
    return in_loc, b, lo, sz


LAST_RESULT = None  # BassKernelResults of the most recent run (for test.py)
TRACE = False  # set True (e.g. by test.py) to capture an NTFF profile


def kernel(a_0, f, g, kernel, steps):
    global _PROGRAM, LAST_RESULT
    from concourse.bass_utils import run_bass_kernel_spmd

    a_0 = np.asarray(a_0, np.float32)
    f = np.asarray(f, np.float32)
    g = np.asarray(g, np.float32)
    W = _conv_matrix(np.asarray(kernel), int(steps))

    in_maps = []
    meta = []
    for core in range(NCORES):
        in_loc, b, lo, sz = _pack_core(core, a_0, f, g, W)
        in_maps.append({"in_loc": in_loc})
        meta.append((b, lo, sz))

    if _PROGRAM is None:
        _PROGRAM = _strip_framework_preamble(_build_program())

    res = run_bass_kernel_spmd(
        _PROGRAM, in_maps, core_ids=list(range(NCORES)), trace=TRACE
    )
    LAST_RESULT = res

    out = np.empty((B, C, N), np.float32)
    for core, (b, lo, sz) in enumerate(meta):
        out[b, lo : lo + sz] = res.results[core]["out_loc"][:sz]
    return out


# revision 38
# speedup vs baseline: 1.0151x; 1.0151x over previous
"""CARFAC cell kernel for 8 TRN2 NeuronCores.

Math: y[b,c,n] is the linear recurrence a[n+1] = f[n+1]*a[n] + g[n+1]
(computed exactly with the DVE's tensor_tensor_scan instruction — the
reference's cumsum-of-logs + triangular-matmul expansion is just a
parallel-friendly expression of the same recurrence), followed by
`steps` rounds of a symmetric-padded 3-tap FIR across channels.

Key identity for the smoothing stage: half-sample symmetric padding
commutes with a symmetric FIR, so applying the 3-tap kernel `steps`
times equals ONE conv with the `steps`-fold self-convolution of the
kernel (17 taps for steps=8) on the reflect-extended signal. That
collapses to a single [C x C] matrix W (banded + boundary-folded),
i.e. one TensorEngine matmul.

Sharding: 8 cores = 2 batches x 4 channel-quarters. Each core loads its
owned ~18 channels plus an 8-channel halo (<=34 rows of f/g), scans the
recurrence for all loaded rows, and applies its [34 x 18] slice of W
(halo selection + reflection encoded host-side in the weights). No
cross-core communication of any kind.

Performance notes (from neuron-profile traces):
- A dynamic DMA's descriptors are processed by ONE SDMA engine
  (~27 GB/s = one SBUF port); the sync, scalar and gpsimd DGE paths
  are independent, so transfers are split across all three.
- The input is loaded in three waves matched to the scan chunks, so the
  scan streams behind the loads instead of waiting for the full input.
- Raw Bass (no Tile, no Block): Tile's tail drain exceeds the HW's
  per-instruction sync-wait cap, and Block's exit all-engine barrier
  costs ~4 us of pure epilogue.
- FP32r single-pass matmuls (4x the fp32 PE rate); end-to-end relative
  error vs the fp32 reference is ~1.9e-4 (fp32 matmuls give ~3e-6 at
  ~+1.5 us — flip FP32R off if a tighter tolerance is ever needed).
"""

import numpy as np

B, C, N = 2, 71, 1024
NCORES = 8
QPB = 4  # channel-quarters per batch element
HALO = 8  # channel reach of the smoothing: steps * (ksz-1)//2
ROWS = 30  # rows loaded per core: own + halo (ownership balanced so every
#            quarter loads the same row count despite one-sided edge halos)
OWN = 22  # max owned output channels per core

_OWN_LO = [0, 22, 36, 50]
_OWN_SZ = [22, 14, 14, 21]

HALF = 512
CL = 256  # scan/matmul pipeline chunk length (Q = 4 chunks)
# Packed input in three DMA waves matched to scan chunks, so the scan
# streams behind the loads (wave 2 carries two chunks: 3 waves is the
# most the per-ring descriptor-generation rate sustains):
#   wave0: [f q0 | g q0 | a0 | W]
#   wave1: [f q1 | g q1]
#   wave2: [f q2 | f q3 | g q2 | g q3]
_A0 = 2 * CL  # 512
_W0 = _A0 + 1
_WAVES = [0, 2 * CL + 1 + OWN, 4 * CL + 1 + OWN, 8 * CL + 1 + OWN]  # boundaries
_FCOL = [0, _WAVES[1], _WAVES[2], _WAVES[2] + CL]
_GCOL = [CL, _WAVES[1] + CL, _WAVES[2] + 2 * CL, _WAVES[2] + 3 * CL]
PACK = _WAVES[3]

FP32R = True  # single-pass PE matmul (4x faster); flip off if accuracy drops
FINAL_WAITS = False  # engines wait for output-DMA completion before halting

_PROGRAM = None


def _build_program():
    import concourse.bass as bass
    import concourse.mybir as mybir

    f32 = mybir.dt.float32
    mm_dt = mybir.dt.float32r if FP32R else f32
    mult, add = mybir.AluOpType.mult, mybir.AluOpType.add
    nc = bass.Bass(enable_partition_id=False)
    in_loc = nc.declare_dram_parameter("in_loc", [ROWS, PACK], f32, isOutput=False)
    out_loc = nc.declare_dram_parameter("out_loc", [OWN, N], f32, isOutput=True)

    Q = 4  # scan/matmul pipeline chunks

    from contextlib import ExitStack

    with ExitStack() as ctx:
        it = ctx.enter_context(nc.sbuf_tensor([ROWS, PACK], f32))
        yt = ctx.enter_context(nc.sbuf_tensor([ROWS, N], mm_dt))
        wr = ctx.enter_context(nc.sbuf_tensor([ROWS, OWN], mm_dt))
        ot = ctx.enter_context(nc.sbuf_tensor([OWN, N], f32))
        ps = [
            ctx.enter_context(nc.psum_tensor(f"ps{q}", [OWN, CL], f32))
            for q in range(4)
        ]
        sem = lambda name: ctx.enter_context(nc.semaphore(name))
        wave_hw = [sem(f"w{w}_hw") for w in range(3)]  # per-wave HWDGE
        wave_sw = [sem(f"w{w}_sw") for w in range(3)]  # per-wave SWDGE
        o_hw = sem("o_hw")  # output stores
        o_sw = sem("o_sw")
        v_sem = sem("v_sem")  # DVE scans
        p_sem = sem("p_sem")  # PE matmuls
        c_sem = sem("c_sem")  # ACT PSUM->SBUF copies
        w_sem = sem("w_sem")  # W staged as FP32r
        w0_hw, w0_sw = wave_hw[0], wave_sw[0]

        a0t = it[:, _A0 : _A0 + 1]
        wt = it[:, _W0 : _W0 + OWN]

        # Input load: three waves, each split across the three DGE paths
        # (rows balanced by measured per-ring gen+transfer rates).
        SPLITS = [("sync", 0, 11), ("scalar", 11, 20), ("gpsimd", 20, 30)]
        for w in range(3):
            c0, c1 = _WAVES[w], _WAVES[w + 1]
            for eng, r0, r1 in SPLITS:
                getattr(nc, eng).dma_start(
                    out=it[r0:r1, c0:c1], in_=in_loc[r0:r1, c0:c1]
                ).then_inc(wave_sw[w] if eng == "gpsimd" else wave_hw[w], 16)

        # ACT: stage W as FP32r (the PE mode needs FP32r producers). Doing
        # this on ACT keeps it off the DVE critical path, and the first
        # ACTIVATE also absorbs the one-time ~1.3us ACT table load before
        # the PSUM-evacuation copies need it.
        nc.scalar.wait_ge(w0_hw, 32)
        nc.scalar.wait_ge(w0_sw, 16)
        nc.scalar.copy(wr[:, :], wt).then_inc(w_sem, 1)

        # DVE: the recurrence scan in Q chunks, chained via
        # initial=prev_out[:, -1:], streaming behind the input waves.
        for q in range(Q):
            t0, t1 = q * CL, (q + 1) * CL
            if q < 3:  # chunk 3's data arrives with wave 2
                nc.vector.wait_ge(wave_hw[q], 32)
                nc.vector.wait_ge(wave_sw[q], 16)
            init = a0t if q == 0 else yt[:, t0 - 1 : t0]
            if q:
                nc.vector.wait_ge(v_sem, q)  # carry readable (race det.)
            nc.vector.tensor_tensor_scan(
                yt[:, t0:t1],
                it[:, _FCOL[q] : _FCOL[q] + CL],
                it[:, _GCOL[q] : _GCOL[q] + CL],
                init,
                op0=mult,
                op1=add,
            ).then_inc(v_sem, 1)

        # PE: one smoothing matmul per chunk.
        nc.tensor.wait_ge(w_sem, 1)
        for q in range(Q):
            nc.tensor.wait_ge(v_sem, q + 1)
            nc.tensor.matmul(
                ps[q][:, :],
                wr[:, :],
                yt[:, q * CL : (q + 1) * CL],
                start=True,
                stop=True,
            ).then_inc(p_sem, 1)

        # ACT: evacuate PSUM per chunk (runs parallel to later scans).
        for q in range(Q):
            nc.scalar.wait_ge(p_sem, q + 1)
            nc.scalar.copy(ot[:, q * CL : (q + 1) * CL], ps[q][:, :]).then_inc(
                c_sem, 1
            )

        # Stores per half. Half 0 on sync+gpsimd (scalar is still copying).
        # Half 1 across all three rings: scalar's own last copy gates it, so
        # its piece needs no cross-engine wake (the wait self-satisfies).
        cols0 = slice(0, HALF)
        for eng, r0, r1 in (("sync", 0, 11), ("gpsimd", 11, 22)):
            e = getattr(nc, eng)
            e.wait_ge(c_sem, 2)
            e.dma_start(out=out_loc[r0:r1, cols0], in_=ot[r0:r1, cols0]).then_inc(
                o_sw if eng == "gpsimd" else o_hw, 16
            )
        cols1 = slice(HALF, N)
        for eng, r0, r1 in (("sync", 0, 8), ("scalar", 8, 15), ("gpsimd", 15, 22)):
            e = getattr(nc, eng)
            e.wait_ge(c_sem, 4)
            e.dma_start(out=out_loc[r0:r1, cols1], in_=ot[r0:r1, cols1]).then_inc(
                o_sw if eng == "gpsimd" else o_hw, 16
            )
        if FINAL_WAITS:
            nc.sync.wait_ge(o_hw, 32)
            nc.gpsimd.wait_ge(o_sw, 32)

    return nc


def _strip_framework_preamble(nc):
    """Drop the framework preamble's const memsets, engine drains and the
    all-engine EVSEM barrier (~4 us on the critical path). Everything in
    this kernel is gated on data semaphores, so engines starting skewed is
    fine. Serialization-level: patches this instance's to_json_bytes."""
    import orjson

    m = nc.to_json()
    for fn in m["functions"]:
        for blk in fn["blocks"]:
            blk["instructions"] = [
                i
                for i in blk["instructions"]
                if not (
                    i.get("opcode") in ("Memset", "Drain")
                    or str(i.get("name", "")).startswith("barrier_")
                )
            ]
    payload = orjson.dumps(m)
    nc.to_json_bytes = lambda: payload
    return nc


def _conv_matrix(kernel: np.ndarray, steps: int) -> np.ndarray:
    """[C, C] matrix equivalent to `steps` rounds of symmetric-pad conv."""
    eff = np.array([1.0], np.float64)
    for _ in range(steps):
        eff = np.convolve(eff, kernel.astype(np.float64))
    h = (len(eff) - 1) // 2
    assert h <= HALO, f"kernel reach {h} exceeds layout halo {HALO}"
    W = np.zeros((C, C), np.float64)
    for c in range(C):
        for d in range(-h, h + 1):
            idx = c + d
            if idx < 0:
                idx = -1 - idx
            if idx >= C:
                idx = 2 * C - 1 - idx
            W[idx, c] += eff[d + h]
    return W.astype(np.float32)


def _pack_core(core: int, a_0, f, g, W):
    """Build one core's packed [ROWS, PACK] input; returns (in_loc, b, lo, sz)."""
    b, q = divmod(core, QPB)
    lo, sz = _OWN_LO[q], _OWN_SZ[q]
    r0 = max(0, lo - HALO)
    r1 = min(C, lo + sz + HALO)
    nr = r1 - r0

    in_loc = np.zeros((ROWS, PACK), np.float32)
    for q in range(4):
        in_loc[:, _FCOL[q] : _FCOL[q] + CL] = 0.5  # benign f for padded rows
        in_loc[:nr, _FCOL[q] : _FCOL[q] + CL] = f[b, r0:r1, q * CL : (q + 1) * CL]
        in_loc[:nr, _GCOL[q] : _GCOL[q] + CL] = g[b, r0:r1, q * CL : (q + 1) * CL]
    in_loc[:nr, _A0] = a_0[b, r0:r1]
    in_loc[:nr, _W0 : _W0 + sz] = W[r0:r1, lo : lo + sz]
    return in_loc, b, lo, sz


LAST_RESULT = None  # BassKernelResults of the most recent run (for test.py)
TRACE = False  # set True (e.g. by test.py) to capture an NTFF profile


def kernel(a_0, f, g, kernel, steps):
    global _PROGRAM, LAST_RESULT
    from concourse.bass_utils import run_bass_kernel_spmd

    a_0 = np.asarray(a_0, np.float32)
    f = np.asarray(f, np.float32)
    g = np.asarray(g, np.float32)
    W = _conv_matrix(np.asarray(kernel), int(steps))

    in_maps = []
    meta = []
    for core in range(NCORES):
        in_loc, b, lo, sz = _pack_core(core, a_0, f, g, W)
        in_maps.append({"in_loc": in_loc})
        meta.append((b, lo, sz))

    if _PROGRAM is None:
        _PROGRAM = _strip_framework_preamble(_build_program())

    res = run_bass_kernel_spmd(
        _PROGRAM, in_maps, core_ids=list(range(NCORES)), trace=TRACE
    )
    LAST_RESULT = res

    out = np.empty((B, C, N), np.float32)
    for core, (b, lo, sz) in enumerate(meta):
        out[b, lo : lo + sz] = res.results[core]["out_loc"][:sz]
    return out
